# revision 20
# baseline (speedup 1.0000x reference)
"""Trainium2 Bass kernel for nn_Mlp_54468775248527.

Ragged masked-mean pooling over two [B, L, D] feature tensors, concat,
3-layer MLP with training-mode BatchNorm (batch stats over full B) + ReLU,
sigmoid head.

Strategy: pure data parallelism — batch B=1024 sharded 128 samples/core
across 8 NeuronCores. The masked mean is computed on TensorE as one
matmul per (sample, 128-feature chunk): stationary = the sample's
[L=128, 128] feature chunk, moving = the sample's [L, 1] mask/len weight
column, output = one PSUM column — which lands the pooled features
directly in transposed [feature, sample] layout for the MLP matmuls.
BatchNorm uses exact full-batch statistics via a tiny [128, 12] f32
AllReduce across the 8 cores (linear bias before train-mode BN cancels,
so fc1_b/fc2_b are dropped). Everything else stays on-chip.
"""
import os
import sys

if '/opt/trn_rl_repo' not in sys.path:
    sys.path.insert(0, '/opt/trn_rl_repo')

import numpy as np

import concourse.bacc as bacc
import concourse.mybir as mybir
import concourse.tile as tile
from concourse.bass_utils import run_bass_kernel_spmd

B, L, D, H = 1024, 128, 768, 256
NCORES = 8
BS = B // NCORES          # 128 samples per core
EPS = 1e-5
f32 = mybir.dt.float32
AX = mybir.AxisListType
AF = mybir.ActivationFunctionType
ALU = mybir.AluOpType

XT_BUFS = 10              # in-flight feature tiles (double-buffer depth)

# debug: 'full' | 'stage1' (masked-sum only) | 'nocc' (no collectives)
_DEBUG_STAGE = os.environ.get('KERNEL_DEBUG_STAGE', 'full')

_NC_CACHE = {}


def _build_nc():
    stage1_only = _DEBUG_STAGE == 'stage1'
    use_cc = _DEBUG_STAGE not in ('nocc', 'stage1')

    nc = bacc.Bacc("TRN2", target_bir_lowering=False, debug=False,
                   enable_asserts=False, num_devices=NCORES)

    ux = nc.dram_tensor("ux", [BS, L, D], f32, kind="ExternalInput")
    hx = nc.dram_tensor("hx", [BS, L, D], f32, kind="ExternalInput")
    uw = nc.dram_tensor("uw", [L, BS], f32, kind="ExternalInput")
    hw = nc.dram_tensor("hw", [L, BS], f32, kind="ExternalInput")
    w1t = nc.dram_tensor("w1t", [2 * D, D], f32, kind="ExternalInput")
    w2t = nc.dram_tensor("w2t", [D, H], f32, kind="ExternalInput")
    w3t = nc.dram_tensor("w3t", [H, 1], f32, kind="ExternalInput")
    bn1g = nc.dram_tensor("bn1g", [128, 6], f32, kind="ExternalInput")
    bn1b = nc.dram_tensor("bn1b", [128, 6], f32, kind="ExternalInput")
    bn2g = nc.dram_tensor("bn2g", [128, 2], f32, kind="ExternalInput")
    bn2b = nc.dram_tensor("bn2b", [128, 2], f32, kind="ExternalInput")
    b3 = nc.dram_tensor("b3", [1, 1], f32, kind="ExternalInput")
    out = nc.dram_tensor("out", [1, BS], f32, kind="ExternalOutput")

    with tile.TileContext(nc) as tc:
        with tc.tile_pool(name="consts", bufs=1) as consts, \
             tc.tile_pool(name="xp", bufs=XT_BUFS) as xp, \
             tc.tile_pool(name="work", bufs=1) as work, \
             tc.tile_pool(name="psp", bufs=1, space="PSUM") as psp, \
             tc.tile_pool(name="dramp", bufs=1, space="DRAM") as dramp:

            uw_sb = consts.tile([L, BS], f32)
            nc.sync.dma_start(uw_sb[:], uw[:])
            hw_sb = consts.tile([L, BS], f32)
            nc.sync.dma_start(hw_sb[:], hw[:])
            w1t_sb = []
            for k in range(12):
                t = consts.tile([128, D], f32, name=f"w1t_sb{k}")
                nc.sync.dma_start(t[:], w1t[k * 128:(k + 1) * 128, :])
                w1t_sb.append(t)
            w2t_sb = []
            for k in range(6):
                t = consts.tile([128, H], f32, name=f"w2t_sb{k}")
                nc.sync.dma_start(t[:], w2t[k * 128:(k + 1) * 128, :])
                w2t_sb.append(t)
            w3t_sb = []
            for c in range(2):
                t = consts.tile([128, 1], f32, name=f"w3t_sb{c}")
                nc.sync.dma_start(t[:], w3t[c * 128:(c + 1) * 128, :])
                w3t_sb.append(t)
            bn1g_sb = consts.tile([128, 6], f32)
            nc.sync.dma_start(bn1g_sb[:], bn1g[:])
            bn1b_sb = consts.tile([128, 6], f32)
            nc.sync.dma_start(bn1b_sb[:], bn1b[:])
            bn2g_sb = consts.tile([128, 2], f32)
            nc.sync.dma_start(bn2g_sb[:], bn2g[:])
            bn2b_sb = consts.tile([128, 2], f32)
            nc.sync.dma_start(bn2b_sb[:], bn2b[:])
            b3_sb = consts.tile([1, 1], f32)
            nc.sync.dma_start(b3_sb[:], b3[:])
            eps_sb = consts.tile([128, 1], f32)
            nc.gpsimd.memset(eps_sb[:], EPS)

            # ---- stage 1: masked mean pooling (transposed output) ----
            # 12 global 128-feature chunks (0..5 user, 6..11 hashtag) packed
            # 4-per-PSUM-bank; sample b is column (chunk%4)*128 + b.
            P0 = psp.tile([128, 512], f32, name="P0")
            P1 = psp.tile([128, 512], f32, name="P1")
            P2 = psp.tile([128, 512], f32, name="P2")
            ptiles = (P0, P1, P2)

            for ti, (xdram, wsb) in enumerate([(ux, uw_sb), (hx, hw_sb)]):
                for b in range(BS):
                    xt = xp.tile([L, D], f32, name="xt", tag="xt")
                    nc.sync.dma_start(xt[:], xdram[b])
                    for c in range(6):
                        g = ti * 6 + c
                        P = ptiles[g // 4]
                        col = (g % 4) * 128 + b
                        nc.tensor.matmul(P[:, col:col + 1],
                                         lhsT=xt[:, c * 128:(c + 1) * 128],
                                         rhs=wsb[:, b:b + 1],
                                         start=True, stop=True)

            xt_sb = work.tile([128, 1536], f32)
            nc.vector.tensor_copy(xt_sb[:, 0:512], P0[:])
            nc.vector.tensor_copy(xt_sb[:, 512:1024], P1[:])
            nc.vector.tensor_copy(xt_sb[:, 1024:1536], P2[:])

            if stage1_only:
                o_dbg = work.tile([1, BS], f32)
                nc.vector.tensor_copy(o_dbg[:], xt_sb[0:1, 0:BS])
                nc.sync.dma_start(out[:], o_dbg[:])
            else:
                _mlp(nc, tc, work, psp, dramp, xt_sb, w1t_sb, w2t_sb, w3t_sb,
                     bn1g_sb, bn1b_sb, bn2g_sb, bn2b_sb, b3_sb, eps_sb, out,
                     use_cc)

    nc.compile()
    return nc


def _allreduce(nc, dramp, work, st, n, name, use_cc):
    """AllReduce an SBUF [128, n] stats tile across the 8 cores."""
    ar_in = dramp.tile([128, n], f32, name=f"{name}_in")
    ar_out = dramp.tile([128, n], f32, name=f"{name}_out")
    nc.sync.dma_start(ar_in[:], st[:])
    if use_cc:
        nc.gpsimd.collective_compute(
            "AllReduce", ALU.add, replica_groups=[list(range(NCORES))],
            ins=[ar_in.opt()], outs=[ar_out.opt()])
    else:
        nc.sync.dma_start(ar_out[:], ar_in[:])
    ast = work.tile([128, n], f32, name=f"{name}_sb")
    nc.sync.dma_start(ast[:], ar_out[:])
    return ast


def _bn_factors(nc, work, ast, gamma, beta, eps_sb, nch, name, nrm):
    """Compute per-feature scale/shift from allreduced (sum, sumsq)."""
    mu = work.tile([128, nch], f32, name=f"mu_{name}")
    nc.scalar.mul(mu[:], ast[:, 0:nch], nrm)
    ex2 = work.tile([128, nch], f32, name=f"ex2_{name}")
    nc.scalar.mul(ex2[:], ast[:, nch:2 * nch], nrm)
    musq = work.tile([128, nch], f32, name=f"musq_{name}")
    nc.vector.tensor_mul(musq[:], mu[:], mu[:])
    var = work.tile([128, nch], f32, name=f"var_{name}")
    nc.vector.tensor_sub(var[:], ex2[:], musq[:])
    std = work.tile([128, nch], f32, name=f"std_{name}")
    nc.scalar.activation(std[:], var[:], AF.Sqrt, bias=eps_sb[:])
    rstd = work.tile([128, nch], f32, name=f"rstd_{name}")
    nc.vector.reciprocal(rstd[:], std[:])
    scl = work.tile([128, nch], f32, name=f"scl_{name}")
    nc.vector.tensor_mul(scl[:], gamma[:], rstd[:])
    tmp = work.tile([128, nch], f32, name=f"tmp_{name}")
    nc.vector.tensor_mul(tmp[:], mu[:], scl[:])
    sh = work.tile([128, nch], f32, name=f"sh_{name}")
    nc.vector.tensor_sub(sh[:], beta[:], tmp[:])
    return scl, sh


def _mlp(nc, tc, work, psp, dramp, xt_sb, w1t_sb, w2t_sb, w3t_sb,
         bn1g_sb, bn1b_sb, bn2g_sb, bn2b_sb, b3_sb, eps_sb, out, use_cc):
    # ---- fc1 (transposed): Z1[j, b] = sum_k w1[j, k] * xT[k, b] ----
    Z1 = psp.tile([128, 768], f32, name="Z1")
    for j in range(6):
        for k in range(12):
            nc.tensor.matmul(Z1[:, j * 128:(j + 1) * 128],
                             lhsT=w1t_sb[k][:, j * 128:(j + 1) * 128],
                             rhs=xt_sb[:, k * 128:(k + 1) * 128],
                             start=(k == 0), stop=(k == 11))

    if _DEBUG_STAGE == 'fc1plain':
        z1p = work.tile([128, 768], f32)
        nc.vector.tensor_copy(z1p[:], Z1[:])
        o_dbg = work.tile([1, BS], f32)
        nc.vector.tensor_copy(o_dbg[:], z1p[0:1, 0:BS])
        nc.sync.dma_start(out[:], o_dbg[:])
        return

    # ---- BN1 stats: PSUM->SBUF copy fused with free-dim sum on ScalarE,
    # sumsq on VectorE from SBUF (PSUM allows only one input port) ----
    z1c = work.tile([128, 768], f32)
    st1 = work.tile([128, 12], f32)
    sq1 = work.tile([128, 128], f32)
    for j in range(6):
        zps = Z1[:, j * 128:(j + 1) * 128]
        zsb = z1c[:, j * 128:(j + 1) * 128]
        nc.scalar.activation(zsb, zps, AF.Identity, accum_out=st1[:, j:j + 1])
        nc.vector.tensor_mul(sq1[:], zsb, zsb)
        nc.vector.reduce_sum(st1[:, 6 + j:7 + j], sq1[:], axis=AX.X)

    if _DEBUG_STAGE == 'fc1':
        o_dbg = work.tile([1, BS], f32)
        nc.vector.tensor_copy(o_dbg[:], z1c[0:1, 0:BS])
        nc.sync.dma_start(out[:], o_dbg[:])
        return

    ast1 = _allreduce(nc, dramp, work, st1, 12, "ar1", use_cc)
    scl1, sh1 = _bn_factors(nc, work, ast1, bn1g_sb, bn1b_sb, eps_sb, 6,
                            "bn1", 1.0 / B)

    if _DEBUG_STAGE == 'bn1':
        o_dbg = work.tile([1, BS], f32)
        nc.vector.tensor_copy(o_dbg[:], scl1[0:1, 0:6].to_broadcast((1, BS)) if False else z1c[0:1, 0:BS])
        nc.vector.tensor_mul(o_dbg[0:1, 0:6], scl1[0:1, 0:6], sh1[0:1, 0:6])
        nc.sync.dma_start(out[:], o_dbg[:])
        return

    x1_sb = work.tile([128, 768], f32)
    for j in range(6):
        nc.scalar.activation(x1_sb[:, j * 128:(j + 1) * 128],
                             z1c[:, j * 128:(j + 1) * 128], AF.Relu,
                             bias=sh1[:, j:j + 1], scale=scl1[:, j:j + 1])

    if _DEBUG_STAGE == 'relu1':
        o_dbg = work.tile([1, BS], f32)
        nc.vector.tensor_copy(o_dbg[:], x1_sb[0:1, 0:BS])
        nc.sync.dma_start(out[:], o_dbg[:])
        return

    # ---- fc2 ----
    Z2 = psp.tile([128, 256], f32, name="Z2")
    for j in range(2):
        for k in range(6):
            nc.tensor.matmul(Z2[:, j * 128:(j + 1) * 128],
                             lhsT=w2t_sb[k][:, j * 128:(j + 1) * 128],
                             rhs=x1_sb[:, k * 128:(k + 1) * 128],
                             start=(k == 0), stop=(k == 5))

    z2c = work.tile([128, 256], f32)
    st2 = work.tile([128, 4], f32)
    sq2 = work.tile([128, 128], f32)
    for j in range(2):
        zps = Z2[:, j * 128:(j + 1) * 128]
        zsb = z2c[:, j * 128:(j + 1) * 128]
        nc.scalar.activation(zsb, zps, AF.Identity, accum_out=st2[:, j:j + 1])
        nc.vector.tensor_mul(sq2[:], zsb, zsb)
        nc.vector.reduce_sum(st2[:, 2 + j:3 + j], sq2[:], axis=AX.X)

    ast2 = _allreduce(nc, dramp, work, st2, 4, "ar2", use_cc)
    scl2, sh2 = _bn_factors(nc, work, ast2, bn2g_sb, bn2b_sb, eps_sb, 2,
                            "bn2", 1.0 / B)

    x2_sb = work.tile([128, 256], f32)
    for j in range(2):
        nc.scalar.activation(x2_sb[:, j * 128:(j + 1) * 128],
                             z2c[:, j * 128:(j + 1) * 128], AF.Relu,
                             bias=sh2[:, j:j + 1], scale=scl2[:, j:j + 1])

    # ---- fc3 + sigmoid ----
    Z3 = psp.tile([1, BS], f32, name="Z3")
    for c in range(2):
        nc.tensor.matmul(Z3[0:1, :], lhsT=w3t_sb[c][:, 0:1],
                         rhs=x2_sb[:, c * 128:(c + 1) * 128],
                         start=(c == 0), stop=(c == 1))
    o_sb = work.tile([1, BS], f32)
    nc.scalar.activation(o_sb[:], Z3[0:1, :], AF.Sigmoid, bias=b3_sb[0:1, 0:1])
    nc.sync.dma_start(out[:], o_sb[:])


def get_nc():
    if 'nc' not in _NC_CACHE:
        _NC_CACHE['nc'] = _build_nc()
    return _NC_CACHE['nc']


def build_in_maps(inputs):
    uf = np.asarray(inputs['user_features'], dtype=np.float32)
    hf = np.asarray(inputs['hashtag_features'], dtype=np.float32)
    ul = np.asarray(inputs['user_lens'])
    hl = np.asarray(inputs['hashtag_lens'])

    ar = np.arange(L)
    uw_full = ((ar[None, :] < ul[:, None]) / ul[:, None]).astype(np.float32)
    hw_full = ((ar[None, :] < hl[:, None]) / hl[:, None]).astype(np.float32)

    w1t = np.ascontiguousarray(np.asarray(inputs['fc1_w'], np.float32).T)
    w2t = np.ascontiguousarray(np.asarray(inputs['fc2_w'], np.float32).T)
    w3t = np.ascontiguousarray(np.asarray(inputs['fc3_w'], np.float32).reshape(1, H).T)
    bn1g = np.ascontiguousarray(np.asarray(inputs['bn1_g'], np.float32).reshape(6, 128).T)
    bn1b = np.ascontiguousarray(np.asarray(inputs['bn1_b'], np.float32).reshape(6, 128).T)
    bn2g = np.ascontiguousarray(np.asarray(inputs['bn2_g'], np.float32).reshape(2, 128).T)
    bn2b = np.ascontiguousarray(np.asarray(inputs['bn2_b'], np.float32).reshape(2, 128).T)
    b3 = np.asarray(inputs['fc3_b'], np.float32).reshape(1, 1)

    in_maps = []
    for c in range(NCORES):
        s = slice(c * BS, (c + 1) * BS)
        in_maps.append({
            'ux': np.ascontiguousarray(uf[s]),
            'hx': np.ascontiguousarray(hf[s]),
            'uw': np.ascontiguousarray(uw_full[s].T),
            'hw': np.ascontiguousarray(hw_full[s].T),
            'w1t': w1t, 'w2t': w2t, 'w3t': w3t,
            'bn1g': bn1g, 'bn1b': bn1b, 'bn2g': bn2g, 'bn2b': bn2b,
            'b3': b3,
        })
    return in_maps


def kernel(**inputs) -> np.ndarray:
    nc = get_nc()
    in_maps = build_in_maps(inputs)
    res = run_bass_kernel_spmd(nc, in_maps, core_ids=list(range(NCORES)))
    out = np.concatenate([res.results[c]['out'][0] for c in range(NCORES)])
    return out.reshape(B, 1).astype(np.float32)


# revision 33
# speedup vs baseline: 1.6721x; 1.6721x over previous
"""Trainium2 Bass kernel for nn_Mlp_54468775248527.

Ragged masked-mean pooling over two [B, L, D] feature tensors, concat,
3-layer MLP with training-mode BatchNorm (batch stats over full B) + ReLU,
sigmoid head.

Strategy: pure data parallelism — batch B=1024 sharded 128 samples/core
across 8 NeuronCores. The masked mean is computed on TensorE as one
matmul per (sample, 128-feature chunk): stationary = the sample's
[L=128, 128] feature chunk, moving = the sample's [L, 1] mask/len weight
column, output = one PSUM column — which lands the pooled features
directly in transposed [feature, sample] layout for the MLP matmuls.
BatchNorm uses exact full-batch statistics via a tiny [128, 12] f32
AllReduce across the 8 cores (linear bias before train-mode BN cancels,
so fc1_b/fc2_b are dropped). Everything else stays on-chip.
"""
import os
import sys

if '/opt/trn_rl_repo' not in sys.path:
    sys.path.insert(0, '/opt/trn_rl_repo')

import numpy as np

import concourse.bacc as bacc
import concourse.mybir as mybir
import concourse.tile as tile
from concourse.bass_utils import run_bass_kernel_spmd

B, L, D, H = 1024, 128, 768, 256
NCORES = 8
BS = B // NCORES          # 128 samples per core
EPS = 1e-5
f32 = mybir.dt.float32
f32r = mybir.dt.float32r   # fp32 layout, single-pass PE mode (fast matmul)
AX = mybir.AxisListType
AF = mybir.ActivationFunctionType
ALU = mybir.AluOpType

XT_BUFS = 10              # in-flight feature tiles (double-buffer depth)

# debug: 'full' | 'stage1' (masked-sum only) | 'nocc' (no collectives)
_DEBUG_STAGE = os.environ.get('KERNEL_DEBUG_STAGE', 'full')

_NC_CACHE = {}


def _build_nc():
    stage1_only = _DEBUG_STAGE == 'stage1'
    use_cc = _DEBUG_STAGE not in ('nocc', 'stage1')

    nc = bacc.Bacc("TRN2", target_bir_lowering=False, debug=False,
                   enable_asserts=False, num_devices=NCORES)

    ux = nc.dram_tensor("ux", [BS, L, D], f32r, kind="ExternalInput")
    hx = nc.dram_tensor("hx", [BS, L, D], f32r, kind="ExternalInput")
    # masks get a 129th all-zero column: fp32r matmuls need moving dim >= 2,
    # so each sample's matmul streams mask columns [b, b+1].
    uw = nc.dram_tensor("uw", [L, BS + 1], f32r, kind="ExternalInput")
    hw = nc.dram_tensor("hw", [L, BS + 1], f32r, kind="ExternalInput")
    w1t = nc.dram_tensor("w1t", [2 * D, D], f32, kind="ExternalInput")
    w2t = nc.dram_tensor("w2t", [D, H], f32, kind="ExternalInput")
    w3t = nc.dram_tensor("w3t", [H, 1], f32, kind="ExternalInput")
    bn1g = nc.dram_tensor("bn1g", [128, 6], f32, kind="ExternalInput")
    bn1b = nc.dram_tensor("bn1b", [128, 6], f32, kind="ExternalInput")
    bn2g = nc.dram_tensor("bn2g", [128, 2], f32, kind="ExternalInput")
    bn2b = nc.dram_tensor("bn2b", [128, 2], f32, kind="ExternalInput")
    b3 = nc.dram_tensor("b3", [1, 1], f32, kind="ExternalInput")
    out = nc.dram_tensor("out", [1, BS], f32, kind="ExternalOutput")

    with tile.TileContext(nc) as tc:
        with tc.tile_pool(name="consts", bufs=1) as consts, \
             tc.tile_pool(name="xp", bufs=XT_BUFS) as xp, \
             tc.tile_pool(name="work", bufs=1) as work, \
             tc.tile_pool(name="psp", bufs=1, space="PSUM") as psp, \
             tc.tile_pool(name="dramp", bufs=1, space="DRAM") as dramp:

            uw_sb = consts.tile([L, BS + 1], f32r)
            nc.sync.dma_start(uw_sb[:], uw[:])
            hw_sb = consts.tile([L, BS + 1], f32r)
            nc.sync.dma_start(hw_sb[:], hw[:])
            w1t_sb = []
            for k in range(12):
                t = consts.tile([128, D], f32, name=f"w1t_sb{k}")
                nc.sync.dma_start(t[:], w1t[k * 128:(k + 1) * 128, :])
                w1t_sb.append(t)
            w2t_sb = []
            for k in range(6):
                t = consts.tile([128, H], f32, name=f"w2t_sb{k}")
                nc.sync.dma_start(t[:], w2t[k * 128:(k + 1) * 128, :])
                w2t_sb.append(t)
            w3t_sb = []
            for c in range(2):
                t = consts.tile([128, 1], f32, name=f"w3t_sb{c}")
                nc.sync.dma_start(t[:], w3t[c * 128:(c + 1) * 128, :])
                w3t_sb.append(t)
            bn1g_sb = consts.tile([128, 6], f32)
            nc.sync.dma_start(bn1g_sb[:], bn1g[:])
            bn1b_sb = consts.tile([128, 6], f32)
            nc.sync.dma_start(bn1b_sb[:], bn1b[:])
            bn2g_sb = consts.tile([128, 2], f32)
            nc.sync.dma_start(bn2g_sb[:], bn2g[:])
            bn2b_sb = consts.tile([128, 2], f32)
            nc.sync.dma_start(bn2b_sb[:], bn2b[:])
            b3_sb = consts.tile([1, 1], f32)
            nc.sync.dma_start(b3_sb[:], b3[:])
            eps_sb = consts.tile([128, 1], f32)
            nc.gpsimd.memset(eps_sb[:], EPS)

            # ---- stage 1: masked mean pooling (transposed output) ----
            # 12 global 128-feature chunks (0..5 user, 6..11 hashtag), 2 per
            # PSUM tile. fp32r matmuls need even PSUM offset + even N, so
            # sample b writes columns (2b, 2b+1): even column = its result,
            # odd column = scratch (mask b+1 against sample b's features,
            # never read). The copy below strides over the even columns.
            ptiles = [psp.tile([128, 512], f32, name=f"P{i}", tag="pp",
                               bufs=6) for i in range(6)]

            for ti, (xdram, wsb) in enumerate([(ux, uw_sb), (hx, hw_sb)]):
                for b in range(BS):
                    xt = xp.tile([L, D], f32r, name="xt", tag="xt")
                    nc.sync.dma_start(xt[:], xdram[b])
                    for c in range(6):
                        g = ti * 6 + c
                        P = ptiles[g // 2]
                        col = (g % 2) * 256 + 2 * b
                        nc.tensor.matmul(P[:, col:col + 2],
                                         lhsT=xt[:, c * 128:(c + 1) * 128],
                                         rhs=wsb[:, b:b + 2],
                                         start=True, stop=True)

            xt_sb = work.tile([128, 1536], f32)
            for g in range(12):
                src = ptiles[g // 2]
                base = (g % 2) * 256
                nc.vector.tensor_copy(
                    xt_sb[:, g * 128:(g + 1) * 128],
                    src[:, base:base + 256].rearrange("p (c two) -> p c two",
                                                      two=2)[:, :, 0])

            if stage1_only:
                o_dbg = work.tile([1, BS], f32)
                nc.vector.tensor_copy(o_dbg[:], xt_sb[0:1, 0:BS])
                nc.sync.dma_start(out[:], o_dbg[:])
            else:
                _mlp(nc, tc, work, psp, dramp, xt_sb, w1t_sb, w2t_sb, w3t_sb,
                     bn1g_sb, bn1b_sb, bn2g_sb, bn2b_sb, b3_sb, eps_sb, out,
                     use_cc)

    nc.compile()
    return nc


def _allreduce(nc, dramp, work, st, n, name, use_cc):
    """AllReduce an SBUF [128, n] stats tile across the 8 cores."""
    ar_in = dramp.tile([128, n], f32, name=f"{name}_in")
    ar_out = dramp.tile([128, n], f32, name=f"{name}_out")
    nc.sync.dma_start(ar_in[:], st[:])
    if use_cc:
        nc.gpsimd.collective_compute(
            "AllReduce", ALU.add, replica_groups=[list(range(NCORES))],
            ins=[ar_in.opt()], outs=[ar_out.opt()])
    else:
        nc.sync.dma_start(ar_out[:], ar_in[:])
    ast = work.tile([128, n], f32, name=f"{name}_sb")
    nc.sync.dma_start(ast[:], ar_out[:])
    return ast


def _bn_factors(nc, work, ast, gamma, beta, eps_sb, nch, name, nrm):
    """Compute per-feature scale/shift from allreduced (sum, sumsq)."""
    mu = work.tile([128, nch], f32, name=f"mu_{name}")
    nc.scalar.mul(mu[:], ast[:, 0:nch], nrm)
    ex2 = work.tile([128, nch], f32, name=f"ex2_{name}")
    nc.scalar.mul(ex2[:], ast[:, nch:2 * nch], nrm)
    musq = work.tile([128, nch], f32, name=f"musq_{name}")
    nc.vector.tensor_mul(musq[:], mu[:], mu[:])
    var = work.tile([128, nch], f32, name=f"var_{name}")
    nc.vector.tensor_sub(var[:], ex2[:], musq[:])
    std = work.tile([128, nch], f32, name=f"std_{name}")
    nc.scalar.activation(std[:], var[:], AF.Sqrt, bias=eps_sb[:])
    rstd = work.tile([128, nch], f32, name=f"rstd_{name}")
    nc.vector.reciprocal(rstd[:], std[:])
    scl = work.tile([128, nch], f32, name=f"scl_{name}")
    nc.vector.tensor_mul(scl[:], gamma[:], rstd[:])
    tmp = work.tile([128, nch], f32, name=f"tmp_{name}")
    nc.vector.tensor_mul(tmp[:], mu[:], scl[:])
    sh = work.tile([128, nch], f32, name=f"sh_{name}")
    nc.vector.tensor_sub(sh[:], beta[:], tmp[:])
    return scl, sh


def _mlp(nc, tc, work, psp, dramp, xt_sb, w1t_sb, w2t_sb, w3t_sb,
         bn1g_sb, bn1b_sb, bn2g_sb, bn2b_sb, b3_sb, eps_sb, out, use_cc):
    # ---- fc1 (transposed): Z1[j, b] = sum_k w1[j, k] * xT[k, b] ----
    # Z1 (3KB) and Z2 (1KB) pack into one 2-bank PSUM tile; Z3 reuses a
    # freed stage-1 slot via the shared "pp" tag.
    Z12 = psp.tile([128, 1024], f32, name="Z12")
    Z1 = Z12[:, 0:768]
    Z2 = Z12[:, 768:1024]
    for j in range(6):
        for k in range(12):
            nc.tensor.matmul(Z1[:, j * 128:(j + 1) * 128],
                             lhsT=w1t_sb[k][:, j * 128:(j + 1) * 128],
                             rhs=xt_sb[:, k * 128:(k + 1) * 128],
                             start=(k == 0), stop=(k == 11))

    if _DEBUG_STAGE == 'fc1plain':
        z1p = work.tile([128, 768], f32)
        nc.vector.tensor_copy(z1p[:], Z1[:])
        o_dbg = work.tile([1, BS], f32)
        nc.vector.tensor_copy(o_dbg[:], z1p[0:1, 0:BS])
        nc.sync.dma_start(out[:], o_dbg[:])
        return

    # ---- BN1 stats: PSUM->SBUF copy fused with free-dim sum on ScalarE,
    # sumsq on VectorE from SBUF (PSUM allows only one input port) ----
    z1c = work.tile([128, 768], f32)
    st1 = work.tile([128, 12], f32)
    sq1 = work.tile([128, 128], f32)
    for j in range(6):
        zps = Z1[:, j * 128:(j + 1) * 128]
        zsb = z1c[:, j * 128:(j + 1) * 128]
        nc.scalar.activation(zsb, zps, AF.Identity, accum_out=st1[:, j:j + 1])
        nc.vector.tensor_mul(sq1[:], zsb, zsb)
        nc.vector.reduce_sum(st1[:, 6 + j:7 + j], sq1[:], axis=AX.X)

    if _DEBUG_STAGE == 'fc1':
        o_dbg = work.tile([1, BS], f32)
        nc.vector.tensor_copy(o_dbg[:], z1c[0:1, 0:BS])
        nc.sync.dma_start(out[:], o_dbg[:])
        return

    ast1 = _allreduce(nc, dramp, work, st1, 12, "ar1", use_cc)
    scl1, sh1 = _bn_factors(nc, work, ast1, bn1g_sb, bn1b_sb, eps_sb, 6,
                            "bn1", 1.0 / B)

    if _DEBUG_STAGE == 'bn1':
        o_dbg = work.tile([1, BS], f32)
        nc.vector.tensor_copy(o_dbg[:], scl1[0:1, 0:6].to_broadcast((1, BS)) if False else z1c[0:1, 0:BS])
        nc.vector.tensor_mul(o_dbg[0:1, 0:6], scl1[0:1, 0:6], sh1[0:1, 0:6])
        nc.sync.dma_start(out[:], o_dbg[:])
        return

    x1_sb = work.tile([128, 768], f32)
    for j in range(6):
        nc.scalar.activation(x1_sb[:, j * 128:(j + 1) * 128],
                             z1c[:, j * 128:(j + 1) * 128], AF.Relu,
                             bias=sh1[:, j:j + 1], scale=scl1[:, j:j + 1])

    if _DEBUG_STAGE == 'relu1':
        o_dbg = work.tile([1, BS], f32)
        nc.vector.tensor_copy(o_dbg[:], x1_sb[0:1, 0:BS])
        nc.sync.dma_start(out[:], o_dbg[:])
        return

    # ---- fc2 ----
    for j in range(2):
        for k in range(6):
            nc.tensor.matmul(Z2[:, j * 128:(j + 1) * 128],
                             lhsT=w2t_sb[k][:, j * 128:(j + 1) * 128],
                             rhs=x1_sb[:, k * 128:(k + 1) * 128],
                             start=(k == 0), stop=(k == 5))

    z2c = work.tile([128, 256], f32)
    st2 = work.tile([128, 4], f32)
    sq2 = work.tile([128, 128], f32)
    for j in range(2):
        zps = Z2[:, j * 128:(j + 1) * 128]
        zsb = z2c[:, j * 128:(j + 1) * 128]
        nc.scalar.activation(zsb, zps, AF.Identity, accum_out=st2[:, j:j + 1])
        nc.vector.tensor_mul(sq2[:], zsb, zsb)
        nc.vector.reduce_sum(st2[:, 2 + j:3 + j], sq2[:], axis=AX.X)

    ast2 = _allreduce(nc, dramp, work, st2, 4, "ar2", use_cc)
    scl2, sh2 = _bn_factors(nc, work, ast2, bn2g_sb, bn2b_sb, eps_sb, 2,
                            "bn2", 1.0 / B)

    x2_sb = work.tile([128, 256], f32)
    for j in range(2):
        nc.scalar.activation(x2_sb[:, j * 128:(j + 1) * 128],
                             z2c[:, j * 128:(j + 1) * 128], AF.Relu,
                             bias=sh2[:, j:j + 1], scale=scl2[:, j:j + 1])

    # ---- fc3 + sigmoid ----
    Z3 = psp.tile([1, BS], f32, name="Z3", tag="pp", bufs=6)
    for c in range(2):
        nc.tensor.matmul(Z3[0:1, :], lhsT=w3t_sb[c][:, 0:1],
                         rhs=x2_sb[:, c * 128:(c + 1) * 128],
                         start=(c == 0), stop=(c == 1))
    o_sb = work.tile([1, BS], f32)
    nc.scalar.activation(o_sb[:], Z3[0:1, :], AF.Sigmoid, bias=b3_sb[0:1, 0:1])
    nc.sync.dma_start(out[:], o_sb[:])


def get_nc():
    if 'nc' not in _NC_CACHE:
        _NC_CACHE['nc'] = _build_nc()
    return _NC_CACHE['nc']


def build_in_maps(inputs):
    uf = np.asarray(inputs['user_features'], dtype=np.float32)
    hf = np.asarray(inputs['hashtag_features'], dtype=np.float32)
    ul = np.asarray(inputs['user_lens'])
    hl = np.asarray(inputs['hashtag_lens'])

    ar = np.arange(L)
    uw_full = ((ar[None, :] < ul[:, None]) / ul[:, None]).astype(np.float32)
    hw_full = ((ar[None, :] < hl[:, None]) / hl[:, None]).astype(np.float32)
    zcol = np.zeros((L, 1), np.float32)

    w1t = np.ascontiguousarray(np.asarray(inputs['fc1_w'], np.float32).T)
    w2t = np.ascontiguousarray(np.asarray(inputs['fc2_w'], np.float32).T)
    w3t = np.ascontiguousarray(np.asarray(inputs['fc3_w'], np.float32).reshape(1, H).T)
    bn1g = np.ascontiguousarray(np.asarray(inputs['bn1_g'], np.float32).reshape(6, 128).T)
    bn1b = np.ascontiguousarray(np.asarray(inputs['bn1_b'], np.float32).reshape(6, 128).T)
    bn2g = np.ascontiguousarray(np.asarray(inputs['bn2_g'], np.float32).reshape(2, 128).T)
    bn2b = np.ascontiguousarray(np.asarray(inputs['bn2_b'], np.float32).reshape(2, 128).T)
    b3 = np.asarray(inputs['fc3_b'], np.float32).reshape(1, 1)

    in_maps = []
    for c in range(NCORES):
        s = slice(c * BS, (c + 1) * BS)
        in_maps.append({
            'ux': np.ascontiguousarray(uf[s]),
            'hx': np.ascontiguousarray(hf[s]),
            'uw': np.ascontiguousarray(np.hstack([uw_full[s].T, zcol])),
            'hw': np.ascontiguousarray(np.hstack([hw_full[s].T, zcol])),
            'w1t': w1t, 'w2t': w2t, 'w3t': w3t,
            'bn1g': bn1g, 'bn1b': bn1b, 'bn2g': bn2g, 'bn2b': bn2b,
            'b3': b3,
        })
    return in_maps


def kernel(**inputs) -> np.ndarray:
    nc = get_nc()
    in_maps = build_in_maps(inputs)
    res = run_bass_kernel_spmd(nc, in_maps, core_ids=list(range(NCORES)))
    out = np.concatenate([res.results[c]['out'][0] for c in range(NCORES)])
    return out.reshape(B, 1).astype(np.float32)
